# revision 19
# baseline (speedup 1.0000x reference)
"""ArcFace-style loss (cos_theta matrix + scalar loss) on 8 TRN2 NeuronCores.

Strategy (vocab / tensor parallel over classes):
  - Each core c owns classes [c*6250, (c+1)*6250), padded to 6272 (=49*128).
  - Host stages per-core W shard TRANSPOSED ([D, Cs] layout) so the
    contraction dim D lands on SBUF partitions for the TensorEngine.
  - Each core: normalize x rows, PE-transpose xn, matmul (f32r) against its
    W^T shard -> cos slice [1024, 6272]; writes slice to DRAM output; a
    fused ScalarE Exp pass + VectorE reduce accumulates per-row
    sum(exp(S*cos)) partials.
  - Label logit: each core gathers cos[b, labels[b]-c_lo] (clamped) from its
    own cos slice in DRAM via indirect DMA, masks rows it does not own.
  - One 8KB AllReduce combines [sumexp_partial | target_partial]; every core
    then computes the identical scalar loss; host reads core 0's.
"""

import math
import os
import sys

import numpy as np

for _p in (
    "/root/.axon_site",
    "/root/.axon_site/_ro/trn_rl_repo",
    "/root/.axon_site/_ro/pypackages",
    "/opt/trn_rl_repo",
):
    if os.path.isdir(_p) and _p not in sys.path:
        sys.path.append(_p)

import concourse.bacc as bacc
import concourse.bass as bass
import concourse.tile as tile
from concourse import mybir
from concourse.bass_utils import run_bass_kernel_spmd

S = 30.0
M = 0.4
EPS = 1e-7
B, D, C = 1024, 512, 50000
NCORES = 8
CS = C // NCORES  # 6250 classes per core
CSP = 6272  # padded to a multiple of 128
P = 128
CHUNK = 512

F32 = mybir.dt.float32
F32R = mybir.dt.float32r
I32 = mybir.dt.int32
AF = mybir.ActivationFunctionType
ALU = mybir.AluOpType
AX = mybir.AxisListType


def build_nc(b=B, d=D, csp=CSP, cs=CS, n_cores=NCORES, use_f32r=True,
             use_collective=True):
    nb = b // P
    nd = d // P
    chunks = []
    c0 = 0
    while c0 < csp:
        cw = min(CHUNK, csp - c0)
        chunks.append((c0, cw))
        c0 += cw
    nch = len(chunks)
    pad_total = float((csp - cs) * n_cores)
    cosM = math.cos(M)
    sinM = math.sin(M)
    mm_dt = F32R if use_f32r else F32

    nc = bacc.Bacc(
        "TRN2",
        target_bir_lowering=False,
        debug=False,
        enable_asserts=False,
        num_devices=n_cores,
    )
    x_d = nc.dram_tensor("x", [b, d], F32, kind="ExternalInput").ap()
    wt_d = nc.dram_tensor("wt", [d, csp], mm_dt, kind="ExternalInput").ap()
    wn_d = nc.dram_tensor("wn", [csp, d], F32, kind="ExternalInput").ap()
    lab_d = nc.dram_tensor("lab", [b], I32, kind="ExternalInput").ap()
    clo_d = nc.dram_tensor("clo", [P, 1], F32, kind="ExternalInput").ap()
    cos_d = nc.dram_tensor("cos", [b, csp], F32, kind="ExternalOutput").ap()
    loss_d = nc.dram_tensor("loss", [1, 1], F32, kind="ExternalOutput").ap()

    eye_const = nc.inline_tensor(np.eye(P, dtype=np.float32), name="eye_const")

    with tile.TileContext(nc) as tc:
        with (
            tc.tile_pool(name="constp", bufs=1) as constp,
            tc.tile_pool(name="xp", bufs=1) as xp,
            tc.tile_pool(name="normp", bufs=2) as normp,
            tc.tile_pool(name="tpsum", bufs=2, space="PSUM") as tpsum,
            tc.tile_pool(name="wp", bufs=3) as wp,
            tc.tile_pool(name="mmpsum", bufs=4, space="PSUM") as mmpsum,
            tc.tile_pool(name="cosp", bufs=6) as cosp,
            tc.tile_pool(name="expp", bufs=4) as expp,
            tc.tile_pool(name="accp", bufs=1) as accp,
            tc.tile_pool(name="tailp", bufs=1) as tailp,
            tc.tile_pool(name="dramp", bufs=1, space="DRAM") as dramp,
        ):
            ident = constp.tile([P, P], F32)
            nc.sync.dma_start(out=ident[:], in_=eye_const.ap())
            ones = constp.tile([P, 1], F32)
            nc.vector.memset(ones[:], 1.0)

            # ---- normalize x rows; build xn^T blocks for the matmul
            x_all = xp.tile([P, nb, d], F32)
            nc.sync.dma_start(out=x_all[:], in_=x_d.rearrange("(t p) d -> p t d", p=P))
            xn_all = xp.tile([P, nb, d], F32)
            xnt = xp.tile([P, nd, b], mm_dt)  # xnt[p, dt, bb] = xn[bb, dt*P + p]
            for bi in range(nb):
                xsq = normp.tile([P, d], F32, tag="xsq")
                ss = normp.tile([P, 1], F32, tag="ss")
                nc.vector.tensor_tensor(
                    out=xsq[:], in0=x_all[:, bi, :], in1=x_all[:, bi, :], op=ALU.mult
                )
                nc.vector.tensor_reduce(ss[:], xsq[:], axis=AX.X, op=ALU.add)
                nrm = normp.tile([P, 1], F32, tag="nrm")
                nc.scalar.activation(nrm[:], ss[:], AF.Sqrt)
                nrmc = normp.tile([P, 1], F32, tag="nrmc")
                nc.vector.tensor_scalar(
                    out=nrmc[:], in0=nrm[:], scalar1=1e-12, scalar2=None, op0=ALU.max
                )
                inv = normp.tile([P, 1], F32, tag="inv")
                nc.vector.reciprocal(inv[:], nrmc[:])
                nc.scalar.activation(
                    xn_all[:, bi, :], x_all[:, bi, :], AF.Copy, scale=inv[:]
                )
                for dt_ in range(nd):
                    pt = tpsum.tile([P, P], F32, tag="tp")
                    nc.tensor.transpose(
                        pt[:], xn_all[:, bi, dt_ * P : (dt_ + 1) * P], ident[:]
                    )
                    nc.vector.tensor_copy(xnt[:, dt_, bi * P : (bi + 1) * P], pt[:])

            # ---- label handling: local row index, ownership mask, W-row gather
            lab_sb = tailp.tile([P, nb], I32)
            nc.sync.dma_start(out=lab_sb[:], in_=lab_d.rearrange("(t p) -> p t", p=P))
            lab_f = tailp.tile([P, nb], F32)
            nc.vector.tensor_copy(lab_f[:], lab_sb[:])
            clo_sb = tailp.tile([P, 1], F32)
            nc.sync.dma_start(out=clo_sb[:], in_=clo_d)
            rel = tailp.tile([P, nb], F32)
            nc.vector.tensor_scalar(
                out=rel[:], in0=lab_f[:], scalar1=clo_sb[:], scalar2=None,
                op0=ALU.subtract,
            )
            idxc_f = tailp.tile([P, nb], F32)
            nc.vector.tensor_scalar(
                out=idxc_f[:], in0=rel[:], scalar1=0.0, scalar2=float(csp - 1),
                op0=ALU.max, op1=ALU.min,
            )
            idxc = tailp.tile([P, nb], I32)
            nc.vector.tensor_copy(idxc[:], idxc_f[:])
            og = tailp.tile([P, nb], F32)
            nc.vector.tensor_scalar(
                out=og[:], in0=rel[:], scalar1=0.0, scalar2=None, op0=ALU.is_ge
            )
            ol = tailp.tile([P, nb], F32)
            nc.vector.tensor_scalar(
                out=ol[:], in0=rel[:], scalar1=float(cs), scalar2=None, op0=ALU.is_lt
            )
            own = tailp.tile([P, nb], F32)
            nc.vector.tensor_tensor(out=own[:], in0=og[:], in1=ol[:], op=ALU.mult)

            # gather W rows for owned labels; dot with xn -> target partial
            tdot = tailp.tile([P, nb], F32)
            for bi in range(nb):
                wlab = normp.tile([P, d], F32, tag="wlab")
                nc.gpsimd.indirect_dma_start(
                    out=wlab[:],
                    out_offset=None,
                    in_=wn_d,
                    in_offset=bass.IndirectOffsetOnAxis(
                        ap=idxc[:, bi : bi + 1], axis=0
                    ),
                )
                dscr = normp.tile([P, d], F32, tag="dscr")
                nc.vector.tensor_tensor(
                    out=dscr[:], in0=xn_all[:, bi, :], in1=wlab[:], op=ALU.mult
                )
                nc.vector.tensor_reduce(
                    tdot[:, bi : bi + 1], dscr[:], axis=AX.X, op=ALU.add
                )
            tpart = tailp.tile([P, nb], F32)
            nc.vector.tensor_tensor(out=tpart[:], in0=tdot[:], in1=own[:], op=ALU.mult)

            # ---- main loop: cos slice + exp partial sums
            sea_parts = accp.tile([P, nb, nch], F32)
            for ci, (c0, cw) in enumerate(chunks):
                w_t = wp.tile([P, nd, CHUNK], mm_dt, tag="w")
                nc.sync.dma_start(
                    out=w_t[:, :, :cw],
                    in_=wt_d.rearrange("(dt p) c -> p dt c", p=P)[:, :, c0 : c0 + cw],
                )
                for bi in range(nb):
                    ps = mmpsum.tile([P, CHUNK], F32, tag="ps")
                    for dt_ in range(nd):
                        nc.tensor.matmul(
                            ps[:, :cw],
                            lhsT=xnt[:, dt_, bi * P : (bi + 1) * P],
                            rhs=w_t[:, dt_, :cw],
                            start=(dt_ == 0),
                            stop=(dt_ == nd - 1),
                        )
                    csb = cosp.tile([P, CHUNK], F32, tag="csb")
                    nc.vector.tensor_copy(csb[:, :cw], ps[:, :cw])
                    nc.sync.dma_start(
                        out=cos_d[bi * P : (bi + 1) * P, c0 : c0 + cw], in_=csb[:, :cw]
                    )
                    ex = expp.tile([P, CHUNK], F32, tag="ex")
                    nc.scalar.activation(ex[:, :cw], ps[:, :cw], AF.Exp, scale=S)
                    nc.vector.tensor_reduce(
                        sea_parts[:, bi, ci : ci + 1], ex[:, :cw], axis=AX.X, op=ALU.add
                    )

            seacc = tailp.tile([P, nb], F32)
            nc.vector.tensor_reduce(seacc[:], sea_parts[:], axis=AX.X, op=ALU.add)

            # ---- AllReduce [sumexp | target]
            arpack = tailp.tile([P, 2 * nb], F32)
            nc.vector.tensor_copy(arpack[:, 0:nb], seacc[:])
            nc.vector.tensor_copy(arpack[:, nb : 2 * nb], tpart[:])
            ar_in = dramp.tile([P, 2 * nb], F32)
            ar_out = dramp.tile([P, 2 * nb], F32)
            nc.sync.dma_start(out=ar_in[:], in_=arpack[:])
            if use_collective:
                nc.gpsimd.collective_compute(
                    "AllReduce",
                    ALU.add,
                    replica_groups=[list(range(n_cores))],
                    ins=[ar_in.opt()],
                    outs=[ar_out.opt()],
                )
            else:
                # bisection mode: no cross-core reduce (loss valid only for
                # data this core owns; cos output unaffected)
                nc.gpsimd.dma_start(out=ar_out[:], in_=ar_in[:])
            arf = tailp.tile([P, 2 * nb], F32)
            nc.sync.dma_start(out=arf[:], in_=ar_out[:])

            # ---- loss tail (identical on every core)
            se_tot = tailp.tile([P, nb], F32)
            nc.vector.tensor_scalar(
                out=se_tot[:], in0=arf[:, 0:nb], scalar1=pad_total, scalar2=None,
                op0=ALU.subtract,
            )
            t_raw = arf[:, nb : 2 * nb]
            t_cl = tailp.tile([P, nb], F32)
            nc.vector.tensor_scalar(
                out=t_cl[:], in0=t_raw, scalar1=-1.0 + EPS, scalar2=1.0 - EPS,
                op0=ALU.max, op1=ALU.min,
            )
            sq = tailp.tile([P, nb], F32)
            nc.vector.tensor_tensor(out=sq[:], in0=t_cl[:], in1=t_cl[:], op=ALU.mult)
            om = tailp.tile([P, nb], F32)
            nc.vector.tensor_scalar(
                out=om[:], in0=sq[:], scalar1=-1.0, scalar2=1.0,
                op0=ALU.mult, op1=ALU.add,
            )
            root = tailp.tile([P, nb], F32)
            nc.scalar.activation(root[:], om[:], AF.Sqrt)
            at = tailp.tile([P, nb], F32)
            nc.vector.tensor_scalar(
                out=at[:], in0=t_cl[:], scalar1=S * cosM, scalar2=None, op0=ALU.mult
            )
            bt = tailp.tile([P, nb], F32)
            nc.vector.tensor_scalar(
                out=bt[:], in0=root[:], scalar1=S * sinM, scalar2=None, op0=ALU.mult
            )
            num = tailp.tile([P, nb], F32)
            nc.vector.tensor_tensor(out=num[:], in0=at[:], in1=bt[:], op=ALU.subtract)
            e_num = tailp.tile([P, nb], F32)
            nc.scalar.activation(e_num[:], num[:], AF.Exp)
            e_st = tailp.tile([P, nb], F32)
            nc.scalar.activation(e_st[:], t_raw, AF.Exp, scale=S)
            den = tailp.tile([P, nb], F32)
            nc.vector.tensor_tensor(out=den[:], in0=e_num[:], in1=se_tot[:], op=ALU.add)
            den2 = tailp.tile([P, nb], F32)
            nc.vector.tensor_tensor(out=den2[:], in0=den[:], in1=e_st[:], op=ALU.subtract)
            lnd = tailp.tile([P, nb], F32)
            nc.scalar.activation(lnd[:], den2[:], AF.Ln)
            lv = tailp.tile([P, nb], F32)
            nc.vector.tensor_tensor(out=lv[:], in0=num[:], in1=lnd[:], op=ALU.subtract)
            row = tailp.tile([P, 1], F32)
            nc.vector.tensor_reduce(row[:], lv[:], axis=AX.X, op=ALU.add)
            pl = tpsum.tile([1, 1], F32, tag="tp")
            nc.tensor.matmul(pl[:], lhsT=row[:], rhs=ones[:], start=True, stop=True)
            lsb = tailp.tile([1, 1], F32)
            nc.scalar.activation(lsb[:], pl[:], AF.Copy, scale=-1.0 / b)
            nc.sync.dma_start(out=loss_d, in_=lsb[:])

    nc.compile()
    return nc


def make_in_maps(x, labels, W, b=B, d=D, csp=CSP, cs=CS, n_cores=NCORES):
    x32 = np.ascontiguousarray(np.asarray(x, dtype=np.float32))
    lab32 = np.ascontiguousarray(np.asarray(labels).astype(np.int32))
    W32 = np.asarray(W, dtype=np.float32)
    in_maps = []
    for i in range(n_cores):
        wp_ = np.zeros((csp, d), dtype=np.float32)
        wp_[:cs] = W32[i * cs : (i + 1) * cs]
        wt = np.ascontiguousarray(wp_.T)
        clo = np.full((P, 1), i * cs, dtype=np.float32)
        in_maps.append({"x": x32, "wt": wt, "wn": wp_, "lab": lab32, "clo": clo})
    return in_maps


_NC_CACHE = {}


def _get_nc():
    if "nc" not in _NC_CACHE:
        _NC_CACHE["nc"] = build_nc()
    return _NC_CACHE["nc"]


def kernel(x, labels, W, _want_time=False):
    nc = _get_nc()
    in_maps = make_in_maps(x, labels, W)
    res = run_bass_kernel_spmd(
        nc, in_maps, core_ids=list(range(NCORES)), trace=_want_time
    )
    cos_full = np.empty((B, C), dtype=np.float32)
    for i in range(NCORES):
        cos_full[:, i * CS : (i + 1) * CS] = res.results[i]["cos"][:, :CS]
    loss = np.float32(res.results[0]["loss"].reshape(())[()])
    if _want_time:
        return (cos_full, loss), res.exec_time_ns
    return (cos_full, loss)


# revision 20
# speedup vs baseline: 1.2414x; 1.2414x over previous
"""ArcFace-style loss (cos_theta matrix + scalar loss) on 8 TRN2 NeuronCores.

Strategy (vocab / tensor parallel over classes):
  - Each core c owns classes [c*6250, (c+1)*6250), padded to 6272 (=49*128).
  - Host stages per-core W shard TRANSPOSED ([D, Cs] layout) so the
    contraction dim D lands on SBUF partitions for the TensorEngine.
  - Each core: normalize x rows, PE-transpose xn, matmul (f32r) against its
    W^T shard -> cos slice [1024, 6272]; writes slice to DRAM output; a
    fused ScalarE Exp pass + VectorE reduce accumulates per-row
    sum(exp(S*cos)) partials.
  - Label logit: each core gathers cos[b, labels[b]-c_lo] (clamped) from its
    own cos slice in DRAM via indirect DMA, masks rows it does not own.
  - One 8KB AllReduce combines [sumexp_partial | target_partial]; every core
    then computes the identical scalar loss; host reads core 0's.
"""

import math
import os
import sys

import numpy as np

for _p in (
    "/root/.axon_site",
    "/root/.axon_site/_ro/trn_rl_repo",
    "/root/.axon_site/_ro/pypackages",
    "/opt/trn_rl_repo",
):
    if os.path.isdir(_p) and _p not in sys.path:
        sys.path.append(_p)

import concourse.bacc as bacc
import concourse.bass as bass
import concourse.tile as tile
from concourse import mybir
from concourse.bass_utils import run_bass_kernel_spmd

S = 30.0
M = 0.4
EPS = 1e-7
B, D, C = 1024, 512, 50000
NCORES = 8
CS = C // NCORES  # 6250 classes per core
CSP = 6272  # padded to a multiple of 128
P = 128
CHUNK = 512

F32 = mybir.dt.float32
F32R = mybir.dt.float32r
I32 = mybir.dt.int32
AF = mybir.ActivationFunctionType
ALU = mybir.AluOpType
AX = mybir.AxisListType


def build_nc(b=B, d=D, csp=CSP, cs=CS, n_cores=NCORES, use_f32r=True,
             use_collective=True):
    nb = b // P
    nd = d // P
    chunks = []
    c0 = 0
    while c0 < csp:
        cw = min(CHUNK, csp - c0)
        chunks.append((c0, cw))
        c0 += cw
    nch = len(chunks)
    pad_total = float((csp - cs) * n_cores)
    cosM = math.cos(M)
    sinM = math.sin(M)
    mm_dt = F32R if use_f32r else F32

    nc = bacc.Bacc(
        "TRN2",
        target_bir_lowering=False,
        debug=False,
        enable_asserts=False,
        num_devices=n_cores,
    )
    x_d = nc.dram_tensor("x", [b, d], F32, kind="ExternalInput").ap()
    wt_d = nc.dram_tensor("wt", [d, csp], mm_dt, kind="ExternalInput").ap()
    wn_d = nc.dram_tensor("wn", [csp, d], F32, kind="ExternalInput").ap()
    lab_d = nc.dram_tensor("lab", [b], I32, kind="ExternalInput").ap()
    clo_d = nc.dram_tensor("clo", [P, 1], F32, kind="ExternalInput").ap()
    cos_d = nc.dram_tensor("cos", [b, csp], F32, kind="ExternalOutput").ap()
    loss_d = nc.dram_tensor("loss", [1, 1], F32, kind="ExternalOutput").ap()

    eye_const = nc.inline_tensor(np.eye(P, dtype=np.float32), name="eye_const")

    with tile.TileContext(nc) as tc:
        with (
            tc.tile_pool(name="constp", bufs=1) as constp,
            tc.tile_pool(name="xp", bufs=1) as xp,
            tc.tile_pool(name="normp", bufs=2) as normp,
            tc.tile_pool(name="tpsum", bufs=2, space="PSUM") as tpsum,
            tc.tile_pool(name="wp", bufs=3) as wp,
            tc.tile_pool(name="mmpsum", bufs=4, space="PSUM") as mmpsum,
            tc.tile_pool(name="cosp", bufs=6) as cosp,
            tc.tile_pool(name="expp", bufs=4) as expp,
            tc.tile_pool(name="accp", bufs=1) as accp,
            tc.tile_pool(name="tailp", bufs=1) as tailp,
            tc.tile_pool(name="dramp", bufs=1, space="DRAM") as dramp,
        ):
            ident = constp.tile([P, P], F32)
            nc.sync.dma_start(out=ident[:], in_=eye_const.ap())
            ones = constp.tile([P, 1], F32)
            nc.vector.memset(ones[:], 1.0)

            # ---- normalize x rows; build xn^T blocks for the matmul
            x_all = xp.tile([P, nb, d], F32)
            nc.sync.dma_start(out=x_all[:], in_=x_d.rearrange("(t p) d -> p t d", p=P))
            xn_all = xp.tile([P, nb, d], F32)
            xnt = xp.tile([P, nd, b], mm_dt)  # xnt[p, dt, bb] = xn[bb, dt*P + p]
            for bi in range(nb):
                xsq = normp.tile([P, d], F32, tag="xsq")
                ss = normp.tile([P, 1], F32, tag="ss")
                nc.vector.tensor_tensor(
                    out=xsq[:], in0=x_all[:, bi, :], in1=x_all[:, bi, :], op=ALU.mult
                )
                nc.vector.tensor_reduce(ss[:], xsq[:], axis=AX.X, op=ALU.add)
                nrm = normp.tile([P, 1], F32, tag="nrm")
                nc.scalar.activation(nrm[:], ss[:], AF.Sqrt)
                nrmc = normp.tile([P, 1], F32, tag="nrmc")
                nc.vector.tensor_scalar(
                    out=nrmc[:], in0=nrm[:], scalar1=1e-12, scalar2=None, op0=ALU.max
                )
                inv = normp.tile([P, 1], F32, tag="inv")
                nc.vector.reciprocal(inv[:], nrmc[:])
                nc.scalar.activation(
                    xn_all[:, bi, :], x_all[:, bi, :], AF.Copy, scale=inv[:]
                )
                for dt_ in range(nd):
                    pt = tpsum.tile([P, P], F32, tag="tp")
                    nc.tensor.transpose(
                        pt[:], xn_all[:, bi, dt_ * P : (dt_ + 1) * P], ident[:]
                    )
                    nc.vector.tensor_copy(xnt[:, dt_, bi * P : (bi + 1) * P], pt[:])

            # ---- main loop: cos slice + exp partial sums (fused on ScalarE)
            # sea_parts[:, bi*nch+ci] = sum_c exp(S*cos) for that tile
            sea_parts = accp.tile([P, nb * nch], F32)

            def load_w_chunk(ci):
                c0, cw = chunks[ci]
                w_t = wp.tile([P, nd, CHUNK], mm_dt, tag="w", name=f"w_t{ci}")
                nc.sync.dma_start(
                    out=w_t[:, :, :cw],
                    in_=wt_d.rearrange("(dt p) c -> p dt c", p=P)[:, :, c0 : c0 + cw],
                )
                return w_t

            w_tiles = {ci: load_w_chunk(ci) for ci in range(min(2, nch))}

            for ci, (c0, cw) in enumerate(chunks):
                w_t = w_tiles.pop(ci) if ci in w_tiles else load_w_chunk(ci)
                for bi in range(nb):
                    ps = mmpsum.tile([P, CHUNK], F32, tag="ps")
                    for dt_ in range(nd):
                        nc.tensor.matmul(
                            ps[:, :cw],
                            lhsT=xnt[:, dt_, bi * P : (bi + 1) * P],
                            rhs=w_t[:, dt_, :cw],
                            start=(dt_ == 0),
                            stop=(dt_ == nd - 1),
                        )
                    csb = cosp.tile([P, CHUNK], F32, tag="csb")
                    nc.vector.tensor_copy(csb[:, :cw], ps[:, :cw])
                    nc.sync.dma_start(
                        out=cos_d[bi * P : (bi + 1) * P, c0 : c0 + cw], in_=csb[:, :cw]
                    )
                    ex = expp.tile([P, CHUNK], F32, tag="ex")
                    k = bi * nch + ci
                    nc.scalar.activation(
                        ex[:, :cw], ps[:, :cw], AF.Exp, scale=S,
                        accum_out=sea_parts[:, k : k + 1],
                    )

            # ---- label handling: local row index, ownership mask, W-row gather
            # (emitted after the main loop so its DVE/DMA work lands in the
            # collective-wait window)
            lab_sb = tailp.tile([P, nb], I32)
            nc.sync.dma_start(out=lab_sb[:], in_=lab_d.rearrange("(t p) -> p t", p=P))
            lab_f = tailp.tile([P, nb], F32)
            nc.vector.tensor_copy(lab_f[:], lab_sb[:])
            clo_sb = tailp.tile([P, 1], F32)
            nc.sync.dma_start(out=clo_sb[:], in_=clo_d)
            rel = tailp.tile([P, nb], F32)
            nc.vector.tensor_scalar(
                out=rel[:], in0=lab_f[:], scalar1=clo_sb[:], scalar2=None,
                op0=ALU.subtract,
            )
            idxc_f = tailp.tile([P, nb], F32)
            nc.vector.tensor_scalar(
                out=idxc_f[:], in0=rel[:], scalar1=0.0, scalar2=float(csp - 1),
                op0=ALU.max, op1=ALU.min,
            )
            idxc = tailp.tile([P, nb], I32)
            nc.vector.tensor_copy(idxc[:], idxc_f[:])
            og = tailp.tile([P, nb], F32)
            nc.vector.tensor_scalar(
                out=og[:], in0=rel[:], scalar1=0.0, scalar2=None, op0=ALU.is_ge
            )
            ol = tailp.tile([P, nb], F32)
            nc.vector.tensor_scalar(
                out=ol[:], in0=rel[:], scalar1=float(cs), scalar2=None, op0=ALU.is_lt
            )
            own = tailp.tile([P, nb], F32)
            nc.vector.tensor_tensor(out=own[:], in0=og[:], in1=ol[:], op=ALU.mult)

            # gather W rows for owned labels; dot with xn -> target partial
            tdot = tailp.tile([P, nb], F32)
            for bi in range(nb):
                wlab = normp.tile([P, d], F32, tag="wlab")
                nc.gpsimd.indirect_dma_start(
                    out=wlab[:],
                    out_offset=None,
                    in_=wn_d,
                    in_offset=bass.IndirectOffsetOnAxis(
                        ap=idxc[:, bi : bi + 1], axis=0
                    ),
                )
                dscr = normp.tile([P, d], F32, tag="dscr")
                nc.vector.tensor_tensor(
                    out=dscr[:], in0=xn_all[:, bi, :], in1=wlab[:], op=ALU.mult
                )
                nc.vector.tensor_reduce(
                    tdot[:, bi : bi + 1], dscr[:], axis=AX.X, op=ALU.add
                )
            tpart = tailp.tile([P, nb], F32)
            nc.vector.tensor_tensor(out=tpart[:], in0=tdot[:], in1=own[:], op=ALU.mult)

            seacc = tailp.tile([P, nb], F32)
            nc.vector.tensor_reduce(
                seacc[:],
                sea_parts[:].rearrange("p (t c) -> p t c", c=nch),
                axis=AX.X,
                op=ALU.add,
            )

            # ---- AllReduce [sumexp | target]
            arpack = tailp.tile([P, 2 * nb], F32)
            nc.vector.tensor_copy(arpack[:, 0:nb], seacc[:])
            nc.vector.tensor_copy(arpack[:, nb : 2 * nb], tpart[:])
            ar_in = dramp.tile([P, 2 * nb], F32)
            ar_out = dramp.tile([P, 2 * nb], F32)
            nc.sync.dma_start(out=ar_in[:], in_=arpack[:])
            if use_collective:
                nc.gpsimd.collective_compute(
                    "AllReduce",
                    ALU.add,
                    replica_groups=[list(range(n_cores))],
                    ins=[ar_in.opt()],
                    outs=[ar_out.opt()],
                )
            else:
                # bisection mode: no cross-core reduce (loss valid only for
                # data this core owns; cos output unaffected)
                nc.gpsimd.dma_start(out=ar_out[:], in_=ar_in[:])
            arf = tailp.tile([P, 2 * nb], F32)
            nc.sync.dma_start(out=arf[:], in_=ar_out[:])

            # ---- loss tail (identical on every core)
            se_tot = tailp.tile([P, nb], F32)
            nc.vector.tensor_scalar(
                out=se_tot[:], in0=arf[:, 0:nb], scalar1=pad_total, scalar2=None,
                op0=ALU.subtract,
            )
            t_raw = arf[:, nb : 2 * nb]
            t_cl = tailp.tile([P, nb], F32)
            nc.vector.tensor_scalar(
                out=t_cl[:], in0=t_raw, scalar1=-1.0 + EPS, scalar2=1.0 - EPS,
                op0=ALU.max, op1=ALU.min,
            )
            sq = tailp.tile([P, nb], F32)
            nc.vector.tensor_tensor(out=sq[:], in0=t_cl[:], in1=t_cl[:], op=ALU.mult)
            om = tailp.tile([P, nb], F32)
            nc.vector.tensor_scalar(
                out=om[:], in0=sq[:], scalar1=-1.0, scalar2=1.0,
                op0=ALU.mult, op1=ALU.add,
            )
            root = tailp.tile([P, nb], F32)
            nc.scalar.activation(root[:], om[:], AF.Sqrt)
            at = tailp.tile([P, nb], F32)
            nc.vector.tensor_scalar(
                out=at[:], in0=t_cl[:], scalar1=S * cosM, scalar2=None, op0=ALU.mult
            )
            bt = tailp.tile([P, nb], F32)
            nc.vector.tensor_scalar(
                out=bt[:], in0=root[:], scalar1=S * sinM, scalar2=None, op0=ALU.mult
            )
            num = tailp.tile([P, nb], F32)
            nc.vector.tensor_tensor(out=num[:], in0=at[:], in1=bt[:], op=ALU.subtract)
            e_num = tailp.tile([P, nb], F32)
            nc.scalar.activation(e_num[:], num[:], AF.Exp)
            e_st = tailp.tile([P, nb], F32)
            nc.scalar.activation(e_st[:], t_raw, AF.Exp, scale=S)
            den = tailp.tile([P, nb], F32)
            nc.vector.tensor_tensor(out=den[:], in0=e_num[:], in1=se_tot[:], op=ALU.add)
            den2 = tailp.tile([P, nb], F32)
            nc.vector.tensor_tensor(out=den2[:], in0=den[:], in1=e_st[:], op=ALU.subtract)
            lnd = tailp.tile([P, nb], F32)
            nc.scalar.activation(lnd[:], den2[:], AF.Ln)
            lv = tailp.tile([P, nb], F32)
            nc.vector.tensor_tensor(out=lv[:], in0=num[:], in1=lnd[:], op=ALU.subtract)
            row = tailp.tile([P, 1], F32)
            nc.vector.tensor_reduce(row[:], lv[:], axis=AX.X, op=ALU.add)
            pl = tpsum.tile([1, 1], F32, tag="tp")
            nc.tensor.matmul(pl[:], lhsT=row[:], rhs=ones[:], start=True, stop=True)
            lsb = tailp.tile([1, 1], F32)
            nc.scalar.activation(lsb[:], pl[:], AF.Copy, scale=-1.0 / b)
            nc.sync.dma_start(out=loss_d, in_=lsb[:])

    nc.compile()
    return nc


def make_in_maps(x, labels, W, b=B, d=D, csp=CSP, cs=CS, n_cores=NCORES):
    x32 = np.ascontiguousarray(np.asarray(x, dtype=np.float32))
    lab32 = np.ascontiguousarray(np.asarray(labels).astype(np.int32))
    W32 = np.asarray(W, dtype=np.float32)
    in_maps = []
    for i in range(n_cores):
        wp_ = np.zeros((csp, d), dtype=np.float32)
        wp_[:cs] = W32[i * cs : (i + 1) * cs]
        wt = np.ascontiguousarray(wp_.T)
        clo = np.full((P, 1), i * cs, dtype=np.float32)
        in_maps.append({"x": x32, "wt": wt, "wn": wp_, "lab": lab32, "clo": clo})
    return in_maps


_NC_CACHE = {}


def _get_nc():
    if "nc" not in _NC_CACHE:
        _NC_CACHE["nc"] = build_nc()
    return _NC_CACHE["nc"]


def kernel(x, labels, W, _want_time=False):
    nc = _get_nc()
    in_maps = make_in_maps(x, labels, W)
    res = run_bass_kernel_spmd(
        nc, in_maps, core_ids=list(range(NCORES)), trace=_want_time
    )
    cos_full = np.empty((B, C), dtype=np.float32)
    for i in range(NCORES):
        cos_full[:, i * CS : (i + 1) * CS] = res.results[i]["cos"][:, :CS]
    loss = np.float32(res.results[0]["loss"].reshape(())[()])
    if _want_time:
        return (cos_full, loss), res.exec_time_ns
    return (cos_full, loss)


# revision 22
# speedup vs baseline: 1.2623x; 1.0168x over previous
"""ArcFace-style loss (cos_theta matrix + scalar loss) on 8 TRN2 NeuronCores.

Strategy (vocab / tensor parallel over classes):
  - Each core c owns classes [c*6250, (c+1)*6250), padded to 6272 (=49*128).
  - Host stages per-core W shard TRANSPOSED ([D, Cs] layout) so the
    contraction dim D lands on SBUF partitions for the TensorEngine.
  - Each core: normalize x rows, PE-transpose xn, matmul (f32r) against its
    W^T shard -> cos slice [1024, 6272]; writes slice to DRAM output; a
    fused ScalarE Exp pass + VectorE reduce accumulates per-row
    sum(exp(S*cos)) partials.
  - Label logit: each core gathers cos[b, labels[b]-c_lo] (clamped) from its
    own cos slice in DRAM via indirect DMA, masks rows it does not own.
  - One 8KB AllReduce combines [sumexp_partial | target_partial]; every core
    then computes the identical scalar loss; host reads core 0's.
"""

import math
import os
import sys

import numpy as np

for _p in (
    "/root/.axon_site",
    "/root/.axon_site/_ro/trn_rl_repo",
    "/root/.axon_site/_ro/pypackages",
    "/opt/trn_rl_repo",
):
    if os.path.isdir(_p) and _p not in sys.path:
        sys.path.append(_p)

import concourse.bacc as bacc
import concourse.bass as bass
import concourse.tile as tile
from concourse import mybir
from concourse.bass_utils import run_bass_kernel_spmd

S = 30.0
M = 0.4
EPS = 1e-7
B, D, C = 1024, 512, 50000
NCORES = 8
CS = C // NCORES  # 6250 classes per core
CSP = 6272  # padded to a multiple of 128
P = 128
CHUNK = 512

F32 = mybir.dt.float32
F32R = mybir.dt.float32r
I32 = mybir.dt.int32
AF = mybir.ActivationFunctionType
ALU = mybir.AluOpType
AX = mybir.AxisListType


def build_nc(b=B, d=D, csp=CSP, cs=CS, n_cores=NCORES, use_f32r=True,
             use_collective=True):
    nb = b // P
    nd = d // P
    chunks = []
    c0 = 0
    while c0 < csp:
        cw = min(CHUNK, csp - c0)
        chunks.append((c0, cw))
        c0 += cw
    nch = len(chunks)
    pad_total = float((csp - cs) * n_cores)
    cosM = math.cos(M)
    sinM = math.sin(M)
    mm_dt = F32R if use_f32r else F32

    nc = bacc.Bacc(
        "TRN2",
        target_bir_lowering=False,
        debug=False,
        enable_asserts=False,
        num_devices=n_cores,
    )
    x_d = nc.dram_tensor("x", [b, d], F32, kind="ExternalInput").ap()
    wt_d = nc.dram_tensor("wt", [d, csp], mm_dt, kind="ExternalInput").ap()
    wn_d = nc.dram_tensor("wn", [csp, d], F32, kind="ExternalInput").ap()
    lab_d = nc.dram_tensor("lab", [b], I32, kind="ExternalInput").ap()
    clo_d = nc.dram_tensor("clo", [P, 1], F32, kind="ExternalInput").ap()
    cos_d = nc.dram_tensor("cos", [b, csp], F32, kind="ExternalOutput").ap()
    loss_d = nc.dram_tensor("loss", [1, 1], F32, kind="ExternalOutput").ap()

    eye_const = nc.inline_tensor(np.eye(P, dtype=np.float32), name="eye_const")

    with tile.TileContext(nc) as tc:
        with (
            tc.tile_pool(name="constp", bufs=1) as constp,
            tc.tile_pool(name="xp", bufs=1) as xp,
            tc.tile_pool(name="normp", bufs=2) as normp,
            tc.tile_pool(name="tpsum", bufs=2, space="PSUM") as tpsum,
            tc.tile_pool(name="wp", bufs=5) as wp,
            tc.tile_pool(name="mmpsum", bufs=4, space="PSUM") as mmpsum,
            tc.tile_pool(name="cosp", bufs=8) as cosp,
            tc.tile_pool(name="expp", bufs=4) as expp,
            tc.tile_pool(name="accp", bufs=1) as accp,
            tc.tile_pool(name="tailp", bufs=1) as tailp,
            tc.tile_pool(name="dramp", bufs=1, space="DRAM") as dramp,
        ):
            ident = constp.tile([P, P], F32)
            nc.sync.dma_start(out=ident[:], in_=eye_const.ap())
            ones = constp.tile([P, 1], F32)
            nc.vector.memset(ones[:], 1.0)

            # ---- normalize x rows; build xn^T blocks for the matmul
            x_all = xp.tile([P, nb, d], F32)
            nc.sync.dma_start(out=x_all[:], in_=x_d.rearrange("(t p) d -> p t d", p=P))
            xn_all = xp.tile([P, nb, d], F32)
            xnt = xp.tile([P, nd, b], mm_dt)  # xnt[p, dt, bb] = xn[bb, dt*P + p]
            for bi in range(nb):
                xsq = normp.tile([P, d], F32, tag="xsq")
                ss = normp.tile([P, 1], F32, tag="ss")
                nc.vector.tensor_tensor(
                    out=xsq[:], in0=x_all[:, bi, :], in1=x_all[:, bi, :], op=ALU.mult
                )
                nc.vector.tensor_reduce(ss[:], xsq[:], axis=AX.X, op=ALU.add)
                nrm = normp.tile([P, 1], F32, tag="nrm")
                nc.scalar.activation(nrm[:], ss[:], AF.Sqrt)
                nrmc = normp.tile([P, 1], F32, tag="nrmc")
                nc.vector.tensor_scalar(
                    out=nrmc[:], in0=nrm[:], scalar1=1e-12, scalar2=None, op0=ALU.max
                )
                inv = normp.tile([P, 1], F32, tag="inv")
                nc.vector.reciprocal(inv[:], nrmc[:])
                nc.scalar.activation(
                    xn_all[:, bi, :], x_all[:, bi, :], AF.Copy, scale=inv[:]
                )
                for dt_ in range(nd):
                    pt = tpsum.tile([P, P], F32, tag="tp")
                    nc.tensor.transpose(
                        pt[:], xn_all[:, bi, dt_ * P : (dt_ + 1) * P], ident[:]
                    )
                    nc.vector.tensor_copy(xnt[:, dt_, bi * P : (bi + 1) * P], pt[:])

            # ---- main loop: cos slice + exp partial sums (fused on ScalarE)
            # sea_parts[:, bi*nch+ci] = sum_c exp(S*cos) for that tile
            sea_parts = accp.tile([P, nb * nch], F32)

            wt_r = wt_d.rearrange("(dt p) c -> p dt c", p=P)

            def load_w_chunk(ci):
                c0, cw = chunks[ci]
                w_t = wp.tile([P, nd, CHUNK], mm_dt, tag="w", name=f"w_t{ci}")
                # per-d-tile DMAs so matmul d=0 can start after 256KB lands
                for dt_ in range(nd):
                    nc.sync.dma_start(
                        out=w_t[:, dt_, :cw],
                        in_=wt_r[:, dt_, c0 : c0 + cw],
                    )
                return w_t

            w_tiles = {ci: load_w_chunk(ci) for ci in range(min(4, nch))}

            for ci, (c0, cw) in enumerate(chunks):
                w_t = w_tiles.pop(ci) if ci in w_tiles else load_w_chunk(ci)
                for bi in range(nb):
                    ps = mmpsum.tile([P, CHUNK], F32, tag="ps")
                    for dt_ in range(nd):
                        nc.tensor.matmul(
                            ps[:, :cw],
                            lhsT=xnt[:, dt_, bi * P : (bi + 1) * P],
                            rhs=w_t[:, dt_, :cw],
                            start=(dt_ == 0),
                            stop=(dt_ == nd - 1),
                        )
                    csb = cosp.tile([P, CHUNK], F32, tag="csb")
                    nc.vector.tensor_copy(csb[:, :cw], ps[:, :cw])
                    nc.sync.dma_start(
                        out=cos_d[bi * P : (bi + 1) * P, c0 : c0 + cw], in_=csb[:, :cw]
                    )
                    ex = expp.tile([P, CHUNK], F32, tag="ex")
                    k = bi * nch + ci
                    nc.scalar.activation(
                        ex[:, :cw], ps[:, :cw], AF.Exp, scale=S,
                        accum_out=sea_parts[:, k : k + 1],
                    )

            # ---- label handling: local row index, ownership mask, W-row gather
            # (emitted after the main loop so its DVE/DMA work lands in the
            # collective-wait window)
            lab_sb = tailp.tile([P, nb], I32)
            nc.sync.dma_start(out=lab_sb[:], in_=lab_d.rearrange("(t p) -> p t", p=P))
            lab_f = tailp.tile([P, nb], F32)
            nc.vector.tensor_copy(lab_f[:], lab_sb[:])
            clo_sb = tailp.tile([P, 1], F32)
            nc.sync.dma_start(out=clo_sb[:], in_=clo_d)
            rel = tailp.tile([P, nb], F32)
            nc.vector.tensor_scalar(
                out=rel[:], in0=lab_f[:], scalar1=clo_sb[:], scalar2=None,
                op0=ALU.subtract,
            )
            idxc_f = tailp.tile([P, nb], F32)
            nc.vector.tensor_scalar(
                out=idxc_f[:], in0=rel[:], scalar1=0.0, scalar2=float(csp - 1),
                op0=ALU.max, op1=ALU.min,
            )
            idxc = tailp.tile([P, nb], I32)
            nc.vector.tensor_copy(idxc[:], idxc_f[:])
            og = tailp.tile([P, nb], F32)
            nc.vector.tensor_scalar(
                out=og[:], in0=rel[:], scalar1=0.0, scalar2=None, op0=ALU.is_ge
            )
            ol = tailp.tile([P, nb], F32)
            nc.vector.tensor_scalar(
                out=ol[:], in0=rel[:], scalar1=float(cs), scalar2=None, op0=ALU.is_lt
            )
            own = tailp.tile([P, nb], F32)
            nc.vector.tensor_tensor(out=own[:], in0=og[:], in1=ol[:], op=ALU.mult)

            # gather W rows for owned labels; dot with xn -> target partial
            tdot = tailp.tile([P, nb], F32)
            for bi in range(nb):
                wlab = normp.tile([P, d], F32, tag="wlab")
                nc.gpsimd.indirect_dma_start(
                    out=wlab[:],
                    out_offset=None,
                    in_=wn_d,
                    in_offset=bass.IndirectOffsetOnAxis(
                        ap=idxc[:, bi : bi + 1], axis=0
                    ),
                )
                dscr = normp.tile([P, d], F32, tag="dscr")
                nc.vector.tensor_tensor(
                    out=dscr[:], in0=xn_all[:, bi, :], in1=wlab[:], op=ALU.mult
                )
                nc.vector.tensor_reduce(
                    tdot[:, bi : bi + 1], dscr[:], axis=AX.X, op=ALU.add
                )
            tpart = tailp.tile([P, nb], F32)
            nc.vector.tensor_tensor(out=tpart[:], in0=tdot[:], in1=own[:], op=ALU.mult)

            seacc = tailp.tile([P, nb], F32)
            nc.vector.tensor_reduce(
                seacc[:],
                sea_parts[:].rearrange("p (t c) -> p t c", c=nch),
                axis=AX.X,
                op=ALU.add,
            )

            # ---- AllReduce [sumexp | target]
            arpack = tailp.tile([P, 2 * nb], F32)
            nc.vector.tensor_copy(arpack[:, 0:nb], seacc[:])
            nc.vector.tensor_copy(arpack[:, nb : 2 * nb], tpart[:])
            ar_in = dramp.tile([P, 2 * nb], F32)
            ar_out = dramp.tile([P, 2 * nb], F32)
            nc.sync.dma_start(out=ar_in[:], in_=arpack[:])
            if use_collective:
                nc.gpsimd.collective_compute(
                    "AllReduce",
                    ALU.add,
                    replica_groups=[list(range(n_cores))],
                    ins=[ar_in.opt()],
                    outs=[ar_out.opt()],
                )
            else:
                # bisection mode: no cross-core reduce (loss valid only for
                # data this core owns; cos output unaffected)
                nc.gpsimd.dma_start(out=ar_out[:], in_=ar_in[:])
            arf = tailp.tile([P, 2 * nb], F32)
            nc.sync.dma_start(out=arf[:], in_=ar_out[:])

            # ---- loss tail (identical on every core)
            se_tot = tailp.tile([P, nb], F32)
            nc.vector.tensor_scalar(
                out=se_tot[:], in0=arf[:, 0:nb], scalar1=pad_total, scalar2=None,
                op0=ALU.subtract,
            )
            t_raw = arf[:, nb : 2 * nb]
            t_cl = tailp.tile([P, nb], F32)
            nc.vector.tensor_scalar(
                out=t_cl[:], in0=t_raw, scalar1=-1.0 + EPS, scalar2=1.0 - EPS,
                op0=ALU.max, op1=ALU.min,
            )
            sq = tailp.tile([P, nb], F32)
            nc.vector.tensor_tensor(out=sq[:], in0=t_cl[:], in1=t_cl[:], op=ALU.mult)
            om = tailp.tile([P, nb], F32)
            nc.vector.tensor_scalar(
                out=om[:], in0=sq[:], scalar1=-1.0, scalar2=1.0,
                op0=ALU.mult, op1=ALU.add,
            )
            root = tailp.tile([P, nb], F32)
            nc.scalar.activation(root[:], om[:], AF.Sqrt)
            at = tailp.tile([P, nb], F32)
            nc.vector.tensor_scalar(
                out=at[:], in0=t_cl[:], scalar1=S * cosM, scalar2=None, op0=ALU.mult
            )
            bt = tailp.tile([P, nb], F32)
            nc.vector.tensor_scalar(
                out=bt[:], in0=root[:], scalar1=S * sinM, scalar2=None, op0=ALU.mult
            )
            num = tailp.tile([P, nb], F32)
            nc.vector.tensor_tensor(out=num[:], in0=at[:], in1=bt[:], op=ALU.subtract)
            e_num = tailp.tile([P, nb], F32)
            nc.scalar.activation(e_num[:], num[:], AF.Exp)
            e_st = tailp.tile([P, nb], F32)
            nc.scalar.activation(e_st[:], t_raw, AF.Exp, scale=S)
            den = tailp.tile([P, nb], F32)
            nc.vector.tensor_tensor(out=den[:], in0=e_num[:], in1=se_tot[:], op=ALU.add)
            den2 = tailp.tile([P, nb], F32)
            nc.vector.tensor_tensor(out=den2[:], in0=den[:], in1=e_st[:], op=ALU.subtract)
            lnd = tailp.tile([P, nb], F32)
            nc.scalar.activation(lnd[:], den2[:], AF.Ln)
            lv = tailp.tile([P, nb], F32)
            nc.vector.tensor_tensor(out=lv[:], in0=num[:], in1=lnd[:], op=ALU.subtract)
            row = tailp.tile([P, 1], F32)
            nc.vector.tensor_reduce(row[:], lv[:], axis=AX.X, op=ALU.add)
            pl = tpsum.tile([1, 1], F32, tag="tp")
            nc.tensor.matmul(pl[:], lhsT=row[:], rhs=ones[:], start=True, stop=True)
            lsb = tailp.tile([1, 1], F32)
            nc.scalar.activation(lsb[:], pl[:], AF.Copy, scale=-1.0 / b)
            nc.sync.dma_start(out=loss_d, in_=lsb[:])

    nc.compile()
    return nc


def make_in_maps(x, labels, W, b=B, d=D, csp=CSP, cs=CS, n_cores=NCORES):
    x32 = np.ascontiguousarray(np.asarray(x, dtype=np.float32))
    lab32 = np.ascontiguousarray(np.asarray(labels).astype(np.int32))
    W32 = np.asarray(W, dtype=np.float32)
    in_maps = []
    for i in range(n_cores):
        wp_ = np.zeros((csp, d), dtype=np.float32)
        wp_[:cs] = W32[i * cs : (i + 1) * cs]
        wt = np.ascontiguousarray(wp_.T)
        clo = np.full((P, 1), i * cs, dtype=np.float32)
        in_maps.append({"x": x32, "wt": wt, "wn": wp_, "lab": lab32, "clo": clo})
    return in_maps


_NC_CACHE = {}


def _get_nc():
    if "nc" not in _NC_CACHE:
        _NC_CACHE["nc"] = build_nc()
    return _NC_CACHE["nc"]


def kernel(x, labels, W, _want_time=False):
    nc = _get_nc()
    in_maps = make_in_maps(x, labels, W)
    res = run_bass_kernel_spmd(
        nc, in_maps, core_ids=list(range(NCORES)), trace=_want_time
    )
    cos_full = np.empty((B, C), dtype=np.float32)
    for i in range(NCORES):
        cos_full[:, i * CS : (i + 1) * CS] = res.results[i]["cos"][:, :CS]
    loss = np.float32(res.results[0]["loss"].reshape(())[()])
    if _want_time:
        return (cos_full, loss), res.exec_time_ns
    return (cos_full, loss)


# revision 25
# speedup vs baseline: 1.3608x; 1.0780x over previous
"""ArcFace-style loss (cos_theta matrix + scalar loss) on 8 TRN2 NeuronCores.

Strategy (vocab / tensor parallel over classes):
  - Each core c owns classes [c*6250, (c+1)*6250), padded to 6272 (=49*128).
  - Host stages per-core W shard TRANSPOSED ([D, Cs] layout) so the
    contraction dim D lands on SBUF partitions for the TensorEngine.
  - Each core: normalize x rows, PE-transpose xn, matmul (f32r) against its
    W^T shard -> cos slice [1024, 6272]; writes slice to DRAM output; a
    fused ScalarE Exp pass + VectorE reduce accumulates per-row
    sum(exp(S*cos)) partials.
  - Label logit: each core gathers cos[b, labels[b]-c_lo] (clamped) from its
    own cos slice in DRAM via indirect DMA, masks rows it does not own.
  - One 8KB AllReduce combines [sumexp_partial | target_partial]; every core
    then computes the identical scalar loss; host reads core 0's.
"""

import math
import os
import sys

import numpy as np

for _p in (
    "/root/.axon_site",
    "/root/.axon_site/_ro/trn_rl_repo",
    "/root/.axon_site/_ro/pypackages",
    "/opt/trn_rl_repo",
):
    if os.path.isdir(_p) and _p not in sys.path:
        sys.path.append(_p)

import concourse.bacc as bacc
import concourse.bass as bass
import concourse.tile as tile
from concourse import mybir
from concourse.bass_utils import run_bass_kernel_spmd

S = 30.0
M = 0.4
EPS = 1e-7
B, D, C = 1024, 512, 50000
NCORES = 8
CS = C // NCORES  # 6250 classes per core
CSP = 6272  # padded to a multiple of 128
P = 128
CHUNK = 512

F32 = mybir.dt.float32
F32R = mybir.dt.float32r
I32 = mybir.dt.int32
AF = mybir.ActivationFunctionType
ALU = mybir.AluOpType
AX = mybir.AxisListType


def build_nc(b=B, d=D, csp=CSP, cs=CS, n_cores=NCORES, use_f32r=True,
             use_collective=True):
    nb = b // P
    nd = d // P
    chunks = []
    c0 = 0
    while c0 < csp:
        cw = min(CHUNK, csp - c0)
        chunks.append((c0, cw))
        c0 += cw
    nch = len(chunks)
    pad_total = float((csp - cs) * n_cores)
    cosM = math.cos(M)
    sinM = math.sin(M)
    mm_dt = F32R if use_f32r else F32

    nc = bacc.Bacc(
        "TRN2",
        target_bir_lowering=False,
        debug=False,
        enable_asserts=False,
        num_devices=n_cores,
    )
    x_d = nc.dram_tensor("x", [b, d], F32, kind="ExternalInput").ap()
    wt_d = nc.dram_tensor("wt", [d, csp], mm_dt, kind="ExternalInput").ap()
    wn_d = nc.dram_tensor("wn", [csp, d], F32, kind="ExternalInput").ap()
    lab_d = nc.dram_tensor("lab", [b], I32, kind="ExternalInput").ap()
    clo_d = nc.dram_tensor("clo", [P, 1], F32, kind="ExternalInput").ap()
    cos_d = nc.dram_tensor("cos", [b, csp], F32, kind="ExternalOutput").ap()
    loss_d = nc.dram_tensor("loss", [1, 1], F32, kind="ExternalOutput").ap()

    eye_const = nc.inline_tensor(np.eye(P, dtype=np.float32), name="eye_const")

    with tile.TileContext(nc) as tc:
        with (
            tc.tile_pool(name="constp", bufs=1) as constp,
            tc.tile_pool(name="xp", bufs=1) as xp,
            tc.tile_pool(name="normp", bufs=2) as normp,
            tc.tile_pool(name="tpsum", bufs=2, space="PSUM") as tpsum,
            tc.tile_pool(name="wp", bufs=5) as wp,
            tc.tile_pool(name="mmpsum", bufs=4, space="PSUM") as mmpsum,
            tc.tile_pool(name="cosp", bufs=8) as cosp,
            tc.tile_pool(name="expp", bufs=4) as expp,
            tc.tile_pool(name="accp", bufs=1) as accp,
            tc.tile_pool(name="tailp", bufs=1) as tailp,
            tc.tile_pool(name="dramp", bufs=1, space="DRAM") as dramp,
        ):
            ident = constp.tile([P, P], F32)
            nc.sync.dma_start(out=ident[:], in_=eye_const.ap())
            ones = constp.tile([P, 1], F32)
            nc.vector.memset(ones[:], 1.0)

            # ---- load x; transpose RAW x on PE immediately (no norm dep);
            # row norms computed in parallel, folded into eviction/exp scales
            x_all = xp.tile([P, nb, d], F32)
            nc.sync.dma_start(out=x_all[:], in_=x_d.rearrange("(t p) d -> p t d", p=P))
            xnt = xp.tile([P, nd, b], mm_dt)  # xnt[p, dt, bb] = x[bb, dt*P + p]
            for bi in range(nb):
                for dt_ in range(nd):
                    pt = tpsum.tile([P, P], F32, tag="tp")
                    nc.tensor.transpose(
                        pt[:], x_all[:, bi, dt_ * P : (dt_ + 1) * P], ident[:]
                    )
                    nc.vector.tensor_copy(xnt[:, dt_, bi * P : (bi + 1) * P], pt[:])
            inv_all = xp.tile([P, nb], F32)  # 1/||x_row||
            for bi in range(nb):
                xsq = normp.tile([P, d], F32, tag="xsq")
                ss = normp.tile([P, 1], F32, tag="ss")
                nc.vector.tensor_tensor(
                    out=xsq[:], in0=x_all[:, bi, :], in1=x_all[:, bi, :], op=ALU.mult
                )
                nc.vector.tensor_reduce(ss[:], xsq[:], axis=AX.X, op=ALU.add)
                nrm = normp.tile([P, 1], F32, tag="nrm")
                nc.scalar.activation(nrm[:], ss[:], AF.Sqrt)
                nrmc = normp.tile([P, 1], F32, tag="nrmc")
                nc.vector.tensor_scalar(
                    out=nrmc[:], in0=nrm[:], scalar1=1e-12, scalar2=None, op0=ALU.max
                )
                nc.vector.reciprocal(inv_all[:, bi : bi + 1], nrmc[:])
            sinv_all = xp.tile([P, nb], F32)  # S/||x_row||
            nc.vector.tensor_scalar(
                out=sinv_all[:], in0=inv_all[:], scalar1=S, scalar2=None, op0=ALU.mult
            )

            # ---- main loop: cos slice + exp partial sums (fused on ScalarE)
            # sea_parts[:, bi*nch+ci] = sum_c exp(S*cos) for that tile
            sea_parts = accp.tile([P, nb * nch], F32)

            wt_r = wt_d.rearrange("(dt p) c -> p dt c", p=P)

            def load_w_chunk(ci):
                c0, cw = chunks[ci]
                w_t = wp.tile([P, nd, CHUNK], mm_dt, tag="w", name=f"w_t{ci}")
                # per-d-tile DMAs so matmul d=0 can start after 256KB lands
                for dt_ in range(nd):
                    nc.sync.dma_start(
                        out=w_t[:, dt_, :cw],
                        in_=wt_r[:, dt_, c0 : c0 + cw],
                    )
                return w_t

            w_tiles = {ci: load_w_chunk(ci) for ci in range(min(4, nch))}

            for ci, (c0, cw) in enumerate(chunks):
                w_t = w_tiles.pop(ci) if ci in w_tiles else load_w_chunk(ci)
                for bi in range(nb):
                    ps = mmpsum.tile([P, CHUNK], F32, tag="ps")
                    for dt_ in range(nd):
                        nc.tensor.matmul(
                            ps[:, :cw],
                            lhsT=xnt[:, dt_, bi * P : (bi + 1) * P],
                            rhs=w_t[:, dt_, :cw],
                            start=(dt_ == 0),
                            stop=(dt_ == nd - 1),
                        )
                    csb = cosp.tile([P, CHUNK], F32, tag="csb")
                    nc.vector.tensor_scalar(
                        out=csb[:, :cw], in0=ps[:, :cw],
                        scalar1=inv_all[:, bi : bi + 1], scalar2=None, op0=ALU.mult,
                    )
                    nc.sync.dma_start(
                        out=cos_d[bi * P : (bi + 1) * P, c0 : c0 + cw], in_=csb[:, :cw]
                    )
                    ex = expp.tile([P, CHUNK], F32, tag="ex")
                    k = bi * nch + ci
                    nc.scalar.activation(
                        ex[:, :cw], ps[:, :cw], AF.Exp,
                        scale=sinv_all[:, bi : bi + 1],
                        accum_out=sea_parts[:, k : k + 1],
                    )

            # ---- label handling: local row index, ownership mask, W-row gather
            # (emitted after the main loop so its DVE/DMA work lands in the
            # collective-wait window)
            lab_sb = tailp.tile([P, nb], I32)
            nc.sync.dma_start(out=lab_sb[:], in_=lab_d.rearrange("(t p) -> p t", p=P))
            lab_f = tailp.tile([P, nb], F32)
            nc.vector.tensor_copy(lab_f[:], lab_sb[:])
            clo_sb = tailp.tile([P, 1], F32)
            nc.sync.dma_start(out=clo_sb[:], in_=clo_d)
            rel = tailp.tile([P, nb], F32)
            nc.vector.tensor_scalar(
                out=rel[:], in0=lab_f[:], scalar1=clo_sb[:], scalar2=None,
                op0=ALU.subtract,
            )
            idxc_f = tailp.tile([P, nb], F32)
            nc.vector.tensor_scalar(
                out=idxc_f[:], in0=rel[:], scalar1=0.0, scalar2=float(csp - 1),
                op0=ALU.max, op1=ALU.min,
            )
            idxc = tailp.tile([P, nb], I32)
            nc.vector.tensor_copy(idxc[:], idxc_f[:])
            og = tailp.tile([P, nb], F32)
            nc.vector.tensor_scalar(
                out=og[:], in0=rel[:], scalar1=0.0, scalar2=None, op0=ALU.is_ge
            )
            ol = tailp.tile([P, nb], F32)
            nc.vector.tensor_scalar(
                out=ol[:], in0=rel[:], scalar1=float(cs), scalar2=None, op0=ALU.is_lt
            )
            own = tailp.tile([P, nb], F32)
            nc.vector.tensor_tensor(out=own[:], in0=og[:], in1=ol[:], op=ALU.mult)

            # gather W rows for owned labels; dot with xn -> target partial
            tdot = tailp.tile([P, nb], F32)
            for bi in range(nb):
                wlab = normp.tile([P, d], F32, tag="wlab")
                nc.gpsimd.indirect_dma_start(
                    out=wlab[:],
                    out_offset=None,
                    in_=wn_d,
                    in_offset=bass.IndirectOffsetOnAxis(
                        ap=idxc[:, bi : bi + 1], axis=0
                    ),
                )
                dscr = normp.tile([P, d], F32, tag="dscr")
                nc.vector.tensor_tensor(
                    out=dscr[:], in0=x_all[:, bi, :], in1=wlab[:], op=ALU.mult
                )
                nc.vector.tensor_reduce(
                    tdot[:, bi : bi + 1], dscr[:], axis=AX.X, op=ALU.add
                )
            tnorm = tailp.tile([P, nb], F32)
            nc.vector.tensor_tensor(out=tnorm[:], in0=tdot[:], in1=inv_all[:], op=ALU.mult)
            tpart = tailp.tile([P, nb], F32)
            nc.vector.tensor_tensor(out=tpart[:], in0=tnorm[:], in1=own[:], op=ALU.mult)

            seacc = tailp.tile([P, nb], F32)
            nc.vector.tensor_reduce(
                seacc[:],
                sea_parts[:].rearrange("p (t c) -> p t c", c=nch),
                axis=AX.X,
                op=ALU.add,
            )

            # ---- AllReduce [sumexp | target]
            arpack = tailp.tile([P, 2 * nb], F32)
            nc.vector.tensor_copy(arpack[:, 0:nb], seacc[:])
            nc.vector.tensor_copy(arpack[:, nb : 2 * nb], tpart[:])
            ar_in = dramp.tile([P, 2 * nb], F32)
            ar_out = dramp.tile([P, 2 * nb], F32)
            nc.sync.dma_start(out=ar_in[:], in_=arpack[:])
            if use_collective:
                nc.gpsimd.collective_compute(
                    "AllReduce",
                    ALU.add,
                    replica_groups=[list(range(n_cores))],
                    ins=[ar_in.opt()],
                    outs=[ar_out.opt()],
                )
            else:
                # bisection mode: no cross-core reduce (loss valid only for
                # data this core owns; cos output unaffected)
                nc.gpsimd.dma_start(out=ar_out[:], in_=ar_in[:])
            arf = tailp.tile([P, 2 * nb], F32)
            nc.sync.dma_start(out=arf[:], in_=ar_out[:])

            # ---- loss tail (identical on every core)
            se_tot = tailp.tile([P, nb], F32)
            nc.vector.tensor_scalar(
                out=se_tot[:], in0=arf[:, 0:nb], scalar1=pad_total, scalar2=None,
                op0=ALU.subtract,
            )
            t_raw = arf[:, nb : 2 * nb]
            t_cl = tailp.tile([P, nb], F32)
            nc.vector.tensor_scalar(
                out=t_cl[:], in0=t_raw, scalar1=-1.0 + EPS, scalar2=1.0 - EPS,
                op0=ALU.max, op1=ALU.min,
            )
            sq = tailp.tile([P, nb], F32)
            nc.vector.tensor_tensor(out=sq[:], in0=t_cl[:], in1=t_cl[:], op=ALU.mult)
            om = tailp.tile([P, nb], F32)
            nc.vector.tensor_scalar(
                out=om[:], in0=sq[:], scalar1=-1.0, scalar2=1.0,
                op0=ALU.mult, op1=ALU.add,
            )
            root = tailp.tile([P, nb], F32)
            nc.scalar.activation(root[:], om[:], AF.Sqrt)
            at = tailp.tile([P, nb], F32)
            nc.vector.tensor_scalar(
                out=at[:], in0=t_cl[:], scalar1=S * cosM, scalar2=None, op0=ALU.mult
            )
            bt = tailp.tile([P, nb], F32)
            nc.vector.tensor_scalar(
                out=bt[:], in0=root[:], scalar1=S * sinM, scalar2=None, op0=ALU.mult
            )
            num = tailp.tile([P, nb], F32)
            nc.vector.tensor_tensor(out=num[:], in0=at[:], in1=bt[:], op=ALU.subtract)
            e_num = tailp.tile([P, nb], F32)
            nc.scalar.activation(e_num[:], num[:], AF.Exp)
            e_st = tailp.tile([P, nb], F32)
            nc.scalar.activation(e_st[:], t_raw, AF.Exp, scale=S)
            den = tailp.tile([P, nb], F32)
            nc.vector.tensor_tensor(out=den[:], in0=e_num[:], in1=se_tot[:], op=ALU.add)
            den2 = tailp.tile([P, nb], F32)
            nc.vector.tensor_tensor(out=den2[:], in0=den[:], in1=e_st[:], op=ALU.subtract)
            lnd = tailp.tile([P, nb], F32)
            nc.scalar.activation(lnd[:], den2[:], AF.Ln)
            lv = tailp.tile([P, nb], F32)
            nc.vector.tensor_tensor(out=lv[:], in0=num[:], in1=lnd[:], op=ALU.subtract)
            row = tailp.tile([P, 1], F32)
            nc.vector.tensor_reduce(row[:], lv[:], axis=AX.X, op=ALU.add)
            pl = tpsum.tile([1, 1], F32, tag="tp")
            nc.tensor.matmul(pl[:], lhsT=row[:], rhs=ones[:], start=True, stop=True)
            lsb = tailp.tile([1, 1], F32)
            nc.scalar.activation(lsb[:], pl[:], AF.Copy, scale=-1.0 / b)
            nc.sync.dma_start(out=loss_d, in_=lsb[:])

    nc.compile()
    return nc


def make_in_maps(x, labels, W, b=B, d=D, csp=CSP, cs=CS, n_cores=NCORES):
    x32 = np.ascontiguousarray(np.asarray(x, dtype=np.float32))
    lab32 = np.ascontiguousarray(np.asarray(labels).astype(np.int32))
    W32 = np.asarray(W, dtype=np.float32)
    in_maps = []
    for i in range(n_cores):
        wp_ = np.zeros((csp, d), dtype=np.float32)
        wp_[:cs] = W32[i * cs : (i + 1) * cs]
        wt = np.ascontiguousarray(wp_.T)
        clo = np.full((P, 1), i * cs, dtype=np.float32)
        in_maps.append({"x": x32, "wt": wt, "wn": wp_, "lab": lab32, "clo": clo})
    return in_maps


_NC_CACHE = {}


def _get_nc():
    if "nc" not in _NC_CACHE:
        _NC_CACHE["nc"] = build_nc()
    return _NC_CACHE["nc"]


def kernel(x, labels, W, _want_time=False):
    nc = _get_nc()
    in_maps = make_in_maps(x, labels, W)
    res = run_bass_kernel_spmd(
        nc, in_maps, core_ids=list(range(NCORES)), trace=_want_time
    )
    cos_full = np.empty((B, C), dtype=np.float32)
    for i in range(NCORES):
        cos_full[:, i * CS : (i + 1) * CS] = res.results[i]["cos"][:, :CS]
    loss = np.float32(res.results[0]["loss"].reshape(())[()])
    if _want_time:
        return (cos_full, loss), res.exec_time_ns
    return (cos_full, loss)
